# revision 31
# baseline (speedup 1.0000x reference)
"""Multi-head causal attention (B=4, S=2048, DX=DM=1024, H=16) on 8 trn2 cores.

Sharding: core c = (batch b = c // 2, head-group g = c % 2).
Megatron-style tensor parallel within a batch: each core computes 8 of the 16
heads (columns g*512:(g+1)*512 of wq/wk/wv, rows of wo) and produces a partial
output [S, DX]; the host sums the two partials per batch and adds bo.

Per-core dataflow (everything transposed so no on-device transposes needed):
  inputs: xqT/xkT/xvT [DX, S] bf16 (host pre-transposed + cast),
          wq/wk/wv [DX, 512] bf16 (wq pre-scaled by 1/sqrt(dk)), wo [512, DX]
  qhT/khT = w.T @ xT          -> [512, S]  (d' on partitions, 2 heads/chunk)
  vh      = xT.T @ wv         -> [S, 512]  (k_s on partitions) + ones column
  scoresT = khT_tile.T @ qhT  -> [k 128, q 512] psum  (K=64, 2 heads row-packed)
  pT      = exp(scoresT)      -> bf16 sbuf (ScalarE), causal mask on diagonal
  zT_aug  = vh_aug.T @ pT     -> [65, q] psum accumulated over k chunks;
            row 64 = softmax denominator (fp32, exact)
  zT      = zT_aug[:64] * (1/denom) broadcast  -> bf16
  outT    = wo.T @ zT         -> [DX, S] fp32, DMA to DRAM; host transposes.
"""

import numpy as np
import ml_dtypes

import concourse.bacc as bacc
import concourse.mybir as mybir
import concourse.tile as tile
from concourse.bass_utils import run_bass_kernel_spmd

BF16 = mybir.dt.bfloat16
F32 = mybir.dt.float32
AF = mybir.ActivationFunctionType
ALU = mybir.AluOpType

P = 128


def build_nc(S=2048, DX=1024, DG=512, QT=512, KC=128, causal=True, dbg=False):
    """Build the single-core Bass program (SPMD across 8 cores)."""
    DK = 64
    NH = DG // DK             # heads per core
    nSq = S // QT             # q tiles
    nCx = DX // P             # contraction chunks for projections
    nMg = DG // P             # d' chunks = head pairs
    nNo = DX // P             # output dx chunks
    kc_per_qt = QT // KC

    nc = bacc.Bacc(
        "TRN2",
        target_bir_lowering=False,
        debug=False,
        enable_asserts=False,
    )

    xq_d = nc.dram_tensor("xqt", [DX, S], BF16, kind="ExternalInput")
    xk_d = nc.dram_tensor("xkt", [DX, S], BF16, kind="ExternalInput")
    xv_d = nc.dram_tensor("xvt", [DX, S], BF16, kind="ExternalInput")
    wq_d = nc.dram_tensor("wq", [DX, DG], BF16, kind="ExternalInput")
    wk_d = nc.dram_tensor("wk", [DX, DG], BF16, kind="ExternalInput")
    wv_d = nc.dram_tensor("wv", [DX, DG], BF16, kind="ExternalInput")
    wo_d = nc.dram_tensor("wo", [DG, DX], BF16, kind="ExternalInput")
    bq_d = nc.dram_tensor("bqs", [P, nMg], F32, kind="ExternalInput")
    bk_d = nc.dram_tensor("bks", [P, nMg], F32, kind="ExternalInput")
    bv_d = nc.dram_tensor("bvr", [1, DG], F32, kind="ExternalInput")
    bo_d = nc.dram_tensor("bor", [P, nNo], F32, kind="ExternalInput")
    out_d = nc.dram_tensor("outT", [DX, S], F32, kind="ExternalOutput")
    if dbg:
        nMg_ = DG // P
        NH_ = DG // 64
        qh_dbg = nc.dram_tensor("qh_dbg", [P, nMg_, S], BF16, kind="ExternalOutput")
        kh_dbg = nc.dram_tensor("kh_dbg", [P, nMg_, S], BF16, kind="ExternalOutput")
        vh_dbg = nc.dram_tensor("vh_dbg", [P, S // KC, NH_, 65], BF16,
                                kind="ExternalOutput")
        zt_dbg = nc.dram_tensor("zt_dbg", [P, nMg_, S], BF16, kind="ExternalOutput")
        pt_dbg = nc.dram_tensor("pt_dbg", [P, 8, QT], BF16, kind="ExternalOutput")

    with tile.TileContext(nc) as tc:
        with (
            tc.tile_pool(name="wpool", bufs=1) as wpool,
            tc.tile_pool(name="wrot", bufs=2) as wrot,
            tc.tile_pool(name="xpool", bufs=1) as xpool,
            tc.tile_pool(name="hpool", bufs=1) as hpool,
            tc.tile_pool(name="ppool", bufs=6) as ppool,
            tc.tile_pool(name="npool", bufs=2) as npool,
            tc.tile_pool(name="opool", bufs=2) as opool,
        ):
            # ---- load weights + biases ----
            # wq/wk/wv/wo rotate 2 slots: wq+wk live for q/k proj, then wv,
            # then wo (loaded during attention, used by outproj)
            wq_t = wrot.tile([P, nCx, DG], BF16, tag="w", name="wq_t")
            wk_t = wrot.tile([P, nCx, DG], BF16, tag="w", name="wk_t")
            nc.sync.dma_start(wq_t[:], wq_d.ap().rearrange("(c p) d -> p c d", p=P))
            nc.sync.dma_start(wk_t[:], wk_d.ap().rearrange("(c p) d -> p c d", p=P))
            bq_t = wpool.tile([P, nMg], F32, tag="bq")
            bk_t = wpool.tile([P, nMg], F32, tag="bk")
            bv_t = wpool.tile([1, DG], F32, tag="bv")
            bo_t = wpool.tile([P, nNo], F32, tag="bo")
            nc.sync.dma_start(bq_t[:], bq_d.ap())
            nc.sync.dma_start(bk_t[:], bk_d.ap())
            nc.sync.dma_start(bv_t[:], bv_d.ap())
            nc.sync.dma_start(bo_t[:], bo_d.ap())
            # broadcast bv across partitions once
            bvb_t = wpool.tile([P, DG], F32, tag="bvb")
            nc.gpsimd.partition_broadcast(bvb_t[:], bv_t[:])

            # ---- projections ----
            qh_t = hpool.tile([P, nMg, S], BF16, tag="qh")
            kh_t = hpool.tile([P, nMg, S], BF16, tag="kh")
            vh_t = hpool.tile([P, S // KC, NH, DK + 1], BF16, tag="vh")
            nc.gpsimd.memset(vh_t[:, :, :, DK:DK + 1], 1.0)

            proj_pool = tc.tile_pool(name="mmps", bufs=2, space="PSUM")
            mmps = proj_pool.__enter__()
            for name, x_d, w_t, h_t, b_t in (
                ("q", xq_d, wq_t, qh_t, bq_t),
                ("k", xk_d, wk_t, kh_t, bk_t),
            ):
                xr = x_d.ap().rearrange("(c p) s -> p c s", p=P)
                for j in range(nSq):
                    x_tj = xpool.tile([P, nCx, QT], BF16, tag="xt",
                                      name=f"x_{name}{j}", bufs=4)
                    nc.sync.dma_start(x_tj[:], xr[:, :, j * QT:(j + 1) * QT])
                    for m in range(nMg):
                        ps = mmps.tile([P, QT], F32, tag="mm", name=f"ps_{name}")
                        for c in range(nCx):
                            nc.tensor.matmul(
                                ps[:],
                                w_t[:, c, m * P:(m + 1) * P],
                                x_tj[:, c, :],
                                start=(c == 0),
                                stop=(c == nCx - 1),
                            )
                        nc.vector.tensor_scalar_add(
                            h_t[:, m, j * QT:(j + 1) * QT], ps[:], b_t[:, m:m + 1]
                        )

            wv_t = wrot.tile([P, nCx, DG], BF16, tag="w", name="wv_t")
            nc.sync.dma_start(wv_t[:], wv_d.ap().rearrange("(c p) d -> p c d", p=P))
            xvr = xv_d.ap().rearrange("(c p) s -> p c s", p=P)
            for j in range(nSq):
                xv_tj = xpool.tile([P, nCx, QT], BF16, tag="xt",
                                   name=f"x_v{j}", bufs=4)
                nc.sync.dma_start(xv_tj[:], xvr[:, :, j * QT:(j + 1) * QT])
                for sml in range(QT // KC):
                    sm = j * (QT // KC) + sml
                    ps = mmps.tile([P, DG], F32, tag="mm", name="ps_v")
                    for c in range(nCx):
                        nc.tensor.matmul(
                            ps[:],
                            xv_tj[:, c, sml * KC:(sml + 1) * KC],
                            wv_t[:, c, :],
                            start=(c == 0),
                            stop=(c == nCx - 1),
                        )
                    nc.vector.tensor_tensor(
                        out=vh_t[:, sm, :, 0:DK],
                        in0=ps[:].rearrange("p (h d) -> p h d", d=DK),
                        in1=bvb_t[:].rearrange("p (h d) -> p h d", d=DK),
                        op=ALU.add,
                    )
            wo_t = wrot.tile([P, nMg, DX], BF16, tag="w", name="wo_t")
            nc.sync.dma_start(wo_t[:], wo_d.ap().rearrange("(c p) d -> p c d", p=P))
            proj_pool.__exit__(None, None, None)

            # ---- attention ----
            att_sps = tc.tile_pool(name="sps", bufs=3, space="PSUM")
            att_zps = tc.tile_pool(name="zps", bufs=2, space="PSUM")
            sps = att_sps.__enter__()
            zps = att_zps.__enter__()
            zt_t = hpool.tile([P, nMg, S], BF16, tag="zt")
            for j in range(nSq):
                nkc_j = (j + 1) * kc_per_qt if causal else S // KC
                # denominator collection tiles: head hh -> dalls[hh // 4],
                # partition slot 32 * (hh % 4) (SBUF APs must start at 0/32/64/96)
                n_dt = (NH + 3) // 4
                dalls = []
                for dti in range(n_dt):
                    dall = npool.tile([P, QT], F32, tag="dall", name="dall")
                    nc.gpsimd.memset(dall[:], 1.0)
                    dalls.append(dall)
                zfs = []
                for hp in range(nMg):
                    zA = zps.tile([DK + 1, QT], F32, tag="z", name="zA")
                    zB = zps.tile([DK + 1, QT], F32, tag="z", name="zB")
                    for i in range(nkc_j):
                        # both heads' score tiles packed side by side in one
                        # 2-bank psum tile -> single wide exp on ScalarE
                        s_ps = sps.tile([P, 2 * QT], F32, tag="s", name="s_ps")
                        for h01 in range(2):
                            lo = h01 * DK
                            nc.tensor.matmul(
                                s_ps[:, h01 * QT:(h01 + 1) * QT],
                                kh_t[lo:lo + DK, hp, i * KC:(i + 1) * KC],
                                qh_t[lo:lo + DK, hp, j * QT:(j + 1) * QT],
                                start=True,
                                stop=True,
                            )
                        pt = ppool.tile([P, 2 * QT], BF16, tag="p", name="pt")
                        nc.scalar.activation(pt[:], s_ps[:], AF.Exp)
                        t = i - j * kc_per_qt
                        if causal and t >= 0:
                            # zero everything left of the diagonal:
                            # keep where q' - k' - t*KC >= 0, over the
                            # column range [0, (t+1)*KC) of each head's half
                            w = (t + 1) * KC
                            for h01 in range(2):
                                nc.gpsimd.affine_select(
                                    out=pt[:, h01 * QT:h01 * QT + w],
                                    in_=pt[:, h01 * QT:h01 * QT + w],
                                    compare_op=ALU.is_ge,
                                    fill=0.0,
                                    base=-(t * KC),
                                    channel_multiplier=-1,
                                    pattern=[[1, w]],
                                )
                        if dbg and j == 0 and hp == 0 and i < 4:
                            for h01 in range(2):
                                nc.sync.dma_start(
                                    pt_dbg.ap()[:, 2 * i + h01, :],
                                    pt[:, h01 * QT:(h01 + 1) * QT])
                        for h01, zp in enumerate((zA, zB)):
                            nc.tensor.matmul(
                                zp[:],
                                vh_t[:, i, 2 * hp + h01, :],
                                pt[:, h01 * QT:(h01 + 1) * QT],
                                start=(i == 0),
                                stop=(i == nkc_j - 1),
                            )
                    for h01, zp in enumerate((zA, zB)):
                        hh = 2 * hp + h01
                        zf = npool.tile([DK + 1, QT], F32, tag="zf", bufs=10)
                        nc.vector.tensor_copy(zf[:], zp[:])
                        sl = 32 * (hh % 4)
                        nc.vector.tensor_copy(
                            dalls[hh // 4][sl:sl + 1, :], zf[DK:DK + 1, :])
                        zfs.append(zf)
                ralls = []
                for dti in range(n_dt):
                    rall = npool.tile([P, QT], F32, tag="rall", name="rall")
                    nc.vector.reciprocal(rall[:], dalls[dti][:])
                    ralls.append(rall)
                for hp in range(nMg):
                    for h01 in range(2):
                        hh = 2 * hp + h01
                        sl = 32 * (hh % 4)
                        # partition_broadcast ucode reads via Q7 core 0: input
                        # must live at partition 0 -> stage through rc
                        rc = npool.tile([1, QT], F32, tag="rc", bufs=2)
                        nc.vector.tensor_copy(rc[:], ralls[hh // 4][sl:sl + 1, :])
                        rb = npool.tile([DK, QT], F32, tag="rb")
                        nc.gpsimd.partition_broadcast(rb[:], rc[:])
                        lo = h01 * DK
                        nc.vector.tensor_tensor(
                            out=zt_t[lo:lo + DK, hp, j * QT:(j + 1) * QT],
                            in0=zfs[hh][0:DK, :],
                            in1=rb[:],
                            op=ALU.mult,
                        )
            if dbg:
                nc.sync.dma_start(qh_dbg.ap(), qh_t[:])
                nc.sync.dma_start(kh_dbg.ap(), kh_t[:])
                nc.sync.dma_start(vh_dbg.ap(), vh_t[:])
                nc.sync.dma_start(zt_dbg.ap(), zt_t[:])

            att_zps.__exit__(None, None, None)
            att_sps.__exit__(None, None, None)

            # ---- output projection ----
            with tc.tile_pool(name="omps", bufs=2, space="PSUM") as omps:
                for n in range(nNo):
                    ob = opool.tile([P, S], F32, tag="ob")
                    for j in range(nSq):
                        ps = omps.tile([P, QT], F32, tag="mm", name="ps_o")
                        for hp in range(nMg):
                            nc.tensor.matmul(
                                ps[:],
                                wo_t[:, hp, n * P:(n + 1) * P],
                                zt_t[:, hp, j * QT:(j + 1) * QT],
                                start=(hp == 0),
                                stop=(hp == nMg - 1),
                            )
                        nc.scalar.activation(
                            ob[:, j * QT:(j + 1) * QT], ps[:], AF.Identity,
                            bias=bo_t[:, n:n + 1])
                    nc.sync.dma_start(out_d.ap()[n * P:(n + 1) * P, :], ob[:])

    nc.compile()
    return nc


def make_in_maps(q, k, v, wq, bq, wk, bk, wv, bv, wo, bo,
                 S=2048, DX=1024, DG=512, n_groups=2):
    """Host-side sharding: slice/transpose/cast per-core inputs."""
    bf16 = ml_dtypes.bfloat16
    B = q.shape[0]
    nMg = DG // P
    nNo = DX // P
    scale = 1.0 / np.sqrt(np.float32(64.0))

    wq_s = (np.asarray(wq, np.float32) * scale)
    bq_s = (np.asarray(bq, np.float32) * scale)

    per_batch = []
    for b in range(B):
        per_batch.append({
            "xqt": np.ascontiguousarray(np.asarray(q[b], np.float32).T).astype(bf16),
            "xkt": np.ascontiguousarray(np.asarray(k[b], np.float32).T).astype(bf16),
            "xvt": np.ascontiguousarray(np.asarray(v[b], np.float32).T).astype(bf16),
        })
    per_group = []
    for g in range(n_groups):
        sl = slice(g * DG, (g + 1) * DG)
        per_group.append({
            "wq": np.ascontiguousarray(wq_s[:, sl]).astype(bf16),
            "wk": np.ascontiguousarray(np.asarray(wk, np.float32)[:, sl]).astype(bf16),
            "wv": np.ascontiguousarray(np.asarray(wv, np.float32)[:, sl]).astype(bf16),
            "wo": np.ascontiguousarray(np.asarray(wo, np.float32)[sl, :]).astype(bf16),
            "bqs": np.ascontiguousarray(bq_s[sl].reshape(nMg, P).T),
            "bks": np.ascontiguousarray(
                np.asarray(bk, np.float32)[sl].reshape(nMg, P).T),
            "bvr": np.asarray(bv, np.float32)[sl].reshape(1, DG).copy(),
            # bo added once per batch: by group 0 only
            "bor": (np.ascontiguousarray(
                        np.asarray(bo, np.float32).reshape(nNo, P).T)
                    if g == 0 else np.zeros((P, nNo), np.float32)),
        })

    in_maps = []
    for b in range(B):
        for g in range(n_groups):
            m = {}
            m.update(per_batch[b])
            m.update(per_group[g])
            in_maps.append(m)
    return in_maps


_NC_CACHE = {}


def _get_nc():
    if "nc" not in _NC_CACHE:
        _NC_CACHE["nc"] = build_nc()
    return _NC_CACHE["nc"]


def run(inputs, trace=False):
    """Run the 8-core kernel; returns (output [B,S,DX] fp32, BassKernelResults)."""
    q = np.asarray(inputs["q"], np.float32)
    B, S, DX = q.shape
    in_maps = make_in_maps(
        q, np.asarray(inputs["k"], np.float32), np.asarray(inputs["v"], np.float32),
        inputs["wq"], inputs["bq"], inputs["wk"], inputs["bk"],
        inputs["wv"], inputs["bv"], inputs["wo"], inputs["bo"],
    )
    nc = _get_nc()
    res = run_bass_kernel_spmd(nc, in_maps, core_ids=list(range(8)), trace=trace)
    out = np.empty((B, S, DX), np.float32)
    for b in range(B):
        acc = res.results[2 * b]["outT"].astype(np.float32) + \
              res.results[2 * b + 1]["outT"].astype(np.float32)
        out[b] = acc.T
    return out, res


def kernel(q, k, v, wq, bq, wk, bk, wv, bv, wo, bo, mask):
    out, _ = run(dict(q=q, k=k, v=v, wq=wq, bq=bq, wk=wk, bk=bk,
                      wv=wv, bv=bv, wo=wo, bo=bo, mask=mask))
    return out


# revision 35
# speedup vs baseline: 1.0381x; 1.0381x over previous
"""Multi-head causal attention (B=4, S=2048, DX=DM=1024, H=16) on 8 trn2 cores.

Sharding: core c = (batch b = c // 2, head-group g = c % 2).
Megatron-style tensor parallel within a batch: each core computes 8 of the 16
heads (columns g*512:(g+1)*512 of wq/wk/wv, rows of wo) and produces a partial
output [S, DX]; the host sums the two partials per batch and adds bo.

Per-core dataflow (everything transposed so no on-device transposes needed):
  inputs: xqT/xkT/xvT [DX, S] bf16 (host pre-transposed + cast),
          wq/wk/wv [DX, 512] bf16 (wq pre-scaled by 1/sqrt(dk)), wo [512, DX]
  qhT/khT = w.T @ xT          -> [512, S]  (d' on partitions, 2 heads/chunk)
  vh      = xT.T @ wv         -> [S, 512]  (k_s on partitions) + ones column
  scoresT = khT_tile.T @ qhT  -> [k 128, q 512] psum  (K=64, 2 heads row-packed)
  pT      = exp(scoresT)      -> bf16 sbuf (ScalarE), causal mask on diagonal
  zT_aug  = vh_aug.T @ pT     -> [65, q] psum accumulated over k chunks;
            row 64 = softmax denominator (fp32, exact)
  zT      = zT_aug[:64] * (1/denom) broadcast  -> bf16
  outT    = wo.T @ zT         -> [DX, S] fp32, DMA to DRAM; host transposes.
"""

import numpy as np
import ml_dtypes

import concourse.bacc as bacc
import concourse.mybir as mybir
import concourse.tile as tile
from concourse.bass_utils import run_bass_kernel_spmd

BF16 = mybir.dt.bfloat16
F32 = mybir.dt.float32
AF = mybir.ActivationFunctionType
ALU = mybir.AluOpType

P = 128


def build_nc(S=2048, DX=1024, DG=512, QT=512, KC=128, causal=True, dbg=False):
    """Build the single-core Bass program (SPMD across 8 cores)."""
    DK = 64
    NH = DG // DK             # heads per core
    nSq = S // QT             # q tiles
    nCx = DX // P             # contraction chunks for projections
    nMg = DG // P             # d' chunks = head pairs
    nNo = DX // P             # output dx chunks
    kc_per_qt = QT // KC

    nc = bacc.Bacc(
        "TRN2",
        target_bir_lowering=False,
        debug=False,
        enable_asserts=False,
    )

    xq_d = nc.dram_tensor("xqt", [DX, S], BF16, kind="ExternalInput")
    xk_d = nc.dram_tensor("xkt", [DX, S], BF16, kind="ExternalInput")
    xv_d = nc.dram_tensor("xvt", [DX, S], BF16, kind="ExternalInput")
    wq_d = nc.dram_tensor("wq", [DX, DG], BF16, kind="ExternalInput")
    wk_d = nc.dram_tensor("wk", [DX, DG], BF16, kind="ExternalInput")
    wv_d = nc.dram_tensor("wv", [DX, DG], BF16, kind="ExternalInput")
    wo_d = nc.dram_tensor("wo", [DG, DX], BF16, kind="ExternalInput")
    bq_d = nc.dram_tensor("bqs", [P, nMg], F32, kind="ExternalInput")
    bk_d = nc.dram_tensor("bks", [P, nMg], F32, kind="ExternalInput")
    bv_d = nc.dram_tensor("bvr", [1, DG], F32, kind="ExternalInput")
    bo_d = nc.dram_tensor("bor", [P, nNo], F32, kind="ExternalInput")
    out_d = nc.dram_tensor("outT", [DX, S], F32, kind="ExternalOutput")
    if dbg:
        nMg_ = DG // P
        NH_ = DG // 64
        qh_dbg = nc.dram_tensor("qh_dbg", [P, nMg_, S], BF16, kind="ExternalOutput")
        kh_dbg = nc.dram_tensor("kh_dbg", [P, nMg_, S], BF16, kind="ExternalOutput")
        vh_dbg = nc.dram_tensor("vh_dbg", [P, S // KC, NH_, 65], BF16,
                                kind="ExternalOutput")
        zt_dbg = nc.dram_tensor("zt_dbg", [P, nMg_, S], BF16, kind="ExternalOutput")
        pt_dbg = nc.dram_tensor("pt_dbg", [P, 8, QT], BF16, kind="ExternalOutput")

    with tile.TileContext(nc) as tc:
        with (
            tc.tile_pool(name="wpool", bufs=1) as wpool,
            tc.tile_pool(name="wrot", bufs=2) as wrot,
            tc.tile_pool(name="xpool", bufs=1) as xpool,
            tc.tile_pool(name="hpool", bufs=1) as hpool,
            tc.tile_pool(name="ppool", bufs=6) as ppool,
            tc.tile_pool(name="npool", bufs=2) as npool,
            tc.tile_pool(name="opool", bufs=2) as opool,
        ):
            # ---- load weights + biases ----
            # wq/wk/wv/wo rotate 2 slots: wq+wk live for q/k proj, then wv,
            # then wo (loaded during attention, used by outproj)
            wq_t = wrot.tile([P, nCx, DG], BF16, tag="w", name="wq_t")
            wk_t = wrot.tile([P, nCx, DG], BF16, tag="w", name="wk_t")
            wqr = wq_d.ap().rearrange("(c p) d -> p c d", p=P)
            # m=0 column slice first: it is the first projection chain's
            # stationary operand, so PE can start ~2 MB earlier
            nc.sync.dma_start(wq_t[:, :, 0:P], wqr[:, :, 0:P])
            xq0_t = xpool.tile([P, nCx, QT], BF16, tag="xt", name="x_q0", bufs=4)
            nc.sync.dma_start(
                xq0_t[:],
                xq_d.ap().rearrange("(c p) s -> p c s", p=P)[:, :, 0:QT])
            if DG > P:
                nc.sync.dma_start(wq_t[:, :, P:], wqr[:, :, P:])
            nc.sync.dma_start(wk_t[:], wk_d.ap().rearrange("(c p) d -> p c d", p=P))
            bq_t = wpool.tile([P, nMg], F32, tag="bq")
            bk_t = wpool.tile([P, nMg], F32, tag="bk")
            bv_t = wpool.tile([1, DG], F32, tag="bv")
            bo_t = wpool.tile([P, nNo], F32, tag="bo")
            nc.sync.dma_start(bq_t[:], bq_d.ap())
            nc.sync.dma_start(bk_t[:], bk_d.ap())
            nc.sync.dma_start(bv_t[:], bv_d.ap())
            nc.sync.dma_start(bo_t[:], bo_d.ap())
            # broadcast bv across partitions once
            bvb_t = wpool.tile([P, DG], F32, tag="bvb")
            nc.gpsimd.partition_broadcast(bvb_t[:], bv_t[:])

            # ---- projections ----
            qh_t = hpool.tile([P, nMg, S], BF16, tag="qh")
            kh_t = hpool.tile([P, nMg, S], BF16, tag="kh")
            vh_t = hpool.tile([P, S // KC, NH, DK + 1], BF16, tag="vh")
            nc.gpsimd.memset(vh_t[:, :, :, DK:DK + 1], 1.0)

            proj_pool = tc.tile_pool(name="mmps", bufs=2, space="PSUM")
            mmps = proj_pool.__enter__()
            for name, x_d, w_t, h_t, b_t in (
                ("q", xq_d, wq_t, qh_t, bq_t),
                ("k", xk_d, wk_t, kh_t, bk_t),
            ):
                xr = x_d.ap().rearrange("(c p) s -> p c s", p=P)
                for j in range(nSq):
                    if name == "q" and j == 0:
                        x_tj = xq0_t
                    else:
                        x_tj = xpool.tile([P, nCx, QT], BF16, tag="xt",
                                          name=f"x_{name}{j}", bufs=4)
                        nc.sync.dma_start(x_tj[:], xr[:, :, j * QT:(j + 1) * QT])
                    for m in range(nMg):
                        ps = mmps.tile([P, QT], F32, tag="mm", name=f"ps_{name}")
                        for c in range(nCx):
                            nc.tensor.matmul(
                                ps[:],
                                w_t[:, c, m * P:(m + 1) * P],
                                x_tj[:, c, :],
                                start=(c == 0),
                                stop=(c == nCx - 1),
                            )
                        nc.vector.tensor_scalar_add(
                            h_t[:, m, j * QT:(j + 1) * QT], ps[:], b_t[:, m:m + 1]
                        )

            wv_t = wrot.tile([P, nCx, DG], BF16, tag="w", name="wv_t")
            nc.sync.dma_start(wv_t[:], wv_d.ap().rearrange("(c p) d -> p c d", p=P))
            xvr = xv_d.ap().rearrange("(c p) s -> p c s", p=P)
            for j in range(nSq):
                xv_tj = xpool.tile([P, nCx, QT], BF16, tag="xt",
                                   name=f"x_v{j}", bufs=4)
                nc.sync.dma_start(xv_tj[:], xvr[:, :, j * QT:(j + 1) * QT])
                for sml in range(QT // KC):
                    sm = j * (QT // KC) + sml
                    ps = mmps.tile([P, DG], F32, tag="mm", name="ps_v")
                    for c in range(nCx):
                        nc.tensor.matmul(
                            ps[:],
                            xv_tj[:, c, sml * KC:(sml + 1) * KC],
                            wv_t[:, c, :],
                            start=(c == 0),
                            stop=(c == nCx - 1),
                        )
                    nc.vector.tensor_tensor(
                        out=vh_t[:, sm, :, 0:DK],
                        in0=ps[:].rearrange("p (h d) -> p h d", d=DK),
                        in1=bvb_t[:].rearrange("p (h d) -> p h d", d=DK),
                        op=ALU.add,
                    )
            wo_t = wrot.tile([P, nMg, DX], BF16, tag="w", name="wo_t")
            nc.sync.dma_start(wo_t[:], wo_d.ap().rearrange("(c p) d -> p c d", p=P))
            proj_pool.__exit__(None, None, None)

            # ---- attention ----
            att_sps = tc.tile_pool(name="sps", bufs=3, space="PSUM")
            att_zps = tc.tile_pool(name="zps", bufs=2, space="PSUM")
            sps = att_sps.__enter__()
            zps = att_zps.__enter__()
            zt_t = hpool.tile([P, nMg, S], BF16, tag="zt")
            for j in range(nSq):
                nkc_j = (j + 1) * kc_per_qt if causal else S // KC
                # denominator collection tiles: head hh -> dalls[hh // 4],
                # partition slot 32 * (hh % 4) (SBUF APs must start at 0/32/64/96)
                n_dt = (NH + 3) // 4
                dalls = []
                for dti in range(n_dt):
                    dall = npool.tile([P, QT], F32, tag="dall", name="dall")
                    nc.gpsimd.memset(dall[:], 1.0)
                    dalls.append(dall)
                zfs = []
                for hp in range(nMg):
                    zA = zps.tile([DK + 1, QT], F32, tag="z", name="zA")
                    zB = zps.tile([DK + 1, QT], F32, tag="z", name="zB")
                    for i in range(nkc_j):
                        # both heads' score tiles packed side by side in one
                        # 2-bank psum tile -> single wide exp on ScalarE
                        s_ps = sps.tile([P, 2 * QT], F32, tag="s", name="s_ps")
                        for h01 in range(2):
                            lo = h01 * DK
                            nc.tensor.matmul(
                                s_ps[:, h01 * QT:(h01 + 1) * QT],
                                kh_t[lo:lo + DK, hp, i * KC:(i + 1) * KC],
                                qh_t[lo:lo + DK, hp, j * QT:(j + 1) * QT],
                                start=True,
                                stop=True,
                            )
                        pt = ppool.tile([P, 2 * QT], BF16, tag="p", name="pt")
                        nc.scalar.activation(pt[:], s_ps[:], AF.Exp)
                        t = i - j * kc_per_qt
                        if causal and t >= 0:
                            # zero everything left of the diagonal:
                            # keep where q' - k' - t*KC >= 0, over the
                            # column range [0, (t+1)*KC) of each head's half
                            w = (t + 1) * KC
                            for h01 in range(2):
                                nc.gpsimd.affine_select(
                                    out=pt[:, h01 * QT:h01 * QT + w],
                                    in_=pt[:, h01 * QT:h01 * QT + w],
                                    compare_op=ALU.is_ge,
                                    fill=0.0,
                                    base=-(t * KC),
                                    channel_multiplier=-1,
                                    pattern=[[1, w]],
                                )
                        if dbg and j == 0 and hp == 0 and i < 4:
                            for h01 in range(2):
                                nc.sync.dma_start(
                                    pt_dbg.ap()[:, 2 * i + h01, :],
                                    pt[:, h01 * QT:(h01 + 1) * QT])
                        for h01, zp in enumerate((zA, zB)):
                            nc.tensor.matmul(
                                zp[:],
                                vh_t[:, i, 2 * hp + h01, :],
                                pt[:, h01 * QT:(h01 + 1) * QT],
                                start=(i == 0),
                                stop=(i == nkc_j - 1),
                            )
                    for h01, zp in enumerate((zA, zB)):
                        hh = 2 * hp + h01
                        zf = npool.tile([DK + 1, QT], F32, tag="zf", bufs=10)
                        nc.vector.tensor_copy(zf[:], zp[:])
                        sl = 32 * (hh % 4)
                        nc.vector.tensor_copy(
                            dalls[hh // 4][sl:sl + 1, :], zf[DK:DK + 1, :])
                        zfs.append(zf)
                ralls = []
                for dti in range(n_dt):
                    rall = npool.tile([P, QT], F32, tag="rall", name="rall")
                    nc.vector.reciprocal(rall[:], dalls[dti][:])
                    ralls.append(rall)
                for hp in range(nMg):
                    for h01 in range(2):
                        hh = 2 * hp + h01
                        sl = 32 * (hh % 4)
                        # partition_broadcast ucode reads via Q7 core 0: input
                        # must live at partition 0 -> stage through rc
                        rc = npool.tile([1, QT], F32, tag="rc", bufs=2)
                        nc.vector.tensor_copy(rc[:], ralls[hh // 4][sl:sl + 1, :])
                        rb = npool.tile([DK, QT], F32, tag="rb")
                        nc.gpsimd.partition_broadcast(rb[:], rc[:])
                        lo = h01 * DK
                        nc.vector.tensor_tensor(
                            out=zt_t[lo:lo + DK, hp, j * QT:(j + 1) * QT],
                            in0=zfs[hh][0:DK, :],
                            in1=rb[:],
                            op=ALU.mult,
                        )
            if dbg:
                nc.sync.dma_start(qh_dbg.ap(), qh_t[:])
                nc.sync.dma_start(kh_dbg.ap(), kh_t[:])
                nc.sync.dma_start(vh_dbg.ap(), vh_t[:])
                nc.sync.dma_start(zt_dbg.ap(), zt_t[:])

            att_zps.__exit__(None, None, None)
            att_sps.__exit__(None, None, None)

            # ---- output projection ----
            with tc.tile_pool(name="omps", bufs=2, space="PSUM") as omps:
                for n in range(nNo):
                    ob = opool.tile([P, S], F32, tag="ob")
                    for j in range(nSq):
                        ps = omps.tile([P, QT], F32, tag="mm", name="ps_o")
                        for hp in range(nMg):
                            nc.tensor.matmul(
                                ps[:],
                                wo_t[:, hp, n * P:(n + 1) * P],
                                zt_t[:, hp, j * QT:(j + 1) * QT],
                                start=(hp == 0),
                                stop=(hp == nMg - 1),
                            )
                        nc.scalar.activation(
                            ob[:, j * QT:(j + 1) * QT], ps[:], AF.Identity,
                            bias=bo_t[:, n:n + 1])
                    nc.sync.dma_start(out_d.ap()[n * P:(n + 1) * P, :], ob[:])

    nc.compile()
    return nc


def make_in_maps(q, k, v, wq, bq, wk, bk, wv, bv, wo, bo,
                 S=2048, DX=1024, DG=512, n_groups=2):
    """Host-side sharding: slice/transpose/cast per-core inputs."""
    bf16 = ml_dtypes.bfloat16
    B = q.shape[0]
    nMg = DG // P
    nNo = DX // P
    scale = 1.0 / np.sqrt(np.float32(64.0))

    wq_s = (np.asarray(wq, np.float32) * scale)
    bq_s = (np.asarray(bq, np.float32) * scale)

    per_batch = []
    for b in range(B):
        per_batch.append({
            "xqt": np.ascontiguousarray(np.asarray(q[b], np.float32).T).astype(bf16),
            "xkt": np.ascontiguousarray(np.asarray(k[b], np.float32).T).astype(bf16),
            "xvt": np.ascontiguousarray(np.asarray(v[b], np.float32).T).astype(bf16),
        })
    per_group = []
    for g in range(n_groups):
        sl = slice(g * DG, (g + 1) * DG)
        per_group.append({
            "wq": np.ascontiguousarray(wq_s[:, sl]).astype(bf16),
            "wk": np.ascontiguousarray(np.asarray(wk, np.float32)[:, sl]).astype(bf16),
            "wv": np.ascontiguousarray(np.asarray(wv, np.float32)[:, sl]).astype(bf16),
            "wo": np.ascontiguousarray(np.asarray(wo, np.float32)[sl, :]).astype(bf16),
            "bqs": np.ascontiguousarray(bq_s[sl].reshape(nMg, P).T),
            "bks": np.ascontiguousarray(
                np.asarray(bk, np.float32)[sl].reshape(nMg, P).T),
            "bvr": np.asarray(bv, np.float32)[sl].reshape(1, DG).copy(),
            # bo added once per batch: by group 0 only
            "bor": (np.ascontiguousarray(
                        np.asarray(bo, np.float32).reshape(nNo, P).T)
                    if g == 0 else np.zeros((P, nNo), np.float32)),
        })

    in_maps = []
    for b in range(B):
        for g in range(n_groups):
            m = {}
            m.update(per_batch[b])
            m.update(per_group[g])
            in_maps.append(m)
    return in_maps


_NC_CACHE = {}


def _get_nc():
    if "nc" not in _NC_CACHE:
        _NC_CACHE["nc"] = build_nc()
    return _NC_CACHE["nc"]


def run(inputs, trace=False):
    """Run the 8-core kernel; returns (output [B,S,DX] fp32, BassKernelResults)."""
    q = np.asarray(inputs["q"], np.float32)
    B, S, DX = q.shape
    in_maps = make_in_maps(
        q, np.asarray(inputs["k"], np.float32), np.asarray(inputs["v"], np.float32),
        inputs["wq"], inputs["bq"], inputs["wk"], inputs["bk"],
        inputs["wv"], inputs["bv"], inputs["wo"], inputs["bo"],
    )
    nc = _get_nc()
    res = run_bass_kernel_spmd(nc, in_maps, core_ids=list(range(8)), trace=trace)
    out = np.empty((B, S, DX), np.float32)
    for b in range(B):
        acc = res.results[2 * b]["outT"].astype(np.float32) + \
              res.results[2 * b + 1]["outT"].astype(np.float32)
        out[b] = acc.T
    return out, res


def kernel(q, k, v, wq, bq, wk, bk, wv, bv, wo, bo, mask):
    out, _ = run(dict(q=q, k=k, v=v, wq=wq, bq=bq, wk=wk, bk=bk,
                      wv=wv, bv=bv, wo=wo, bo=bo, mask=mask))
    return out
